# revision 24
# baseline (speedup 1.0000x reference)
"""Trainium2 Bass kernel for nn_BilinearModule (16,256,64,64 bilinear pooling).

Math (per image):
  y   = relu(bn1(w1 @ x + b1))                       # (32, 4096)
  packed[t] = y[r_t] * y[c_t]  for 528 lower-tri pairs
  out = relu(bn2(w2 @ packed + b2))                  # (256, 4096)

Strategy (pure data parallel over batch, 2 images per core, 8 cores):
  - all matmul operands bf16 (x cast host-side, halves the input DMA);
    fp32 PSUM accumulation and fp32 BN math keep the error ~5e-3.
  - mm1 with M-replicated weights -> psum; fused BN1+ReLU on ACT -> yrep bf16
    (4 identical copies of the 32 channels across 128 partitions).
  - The 528 pair-products are covered by 17 channel *rotations* r=0..16:
    rotation r yields pairs {c, (c+r)%32} = diag r plus diag 32-r, all
    distinct (r=16 half-duplicated). Rotated tiles are produced by 5 K=32
    permutation matmuls (4 rotations per tile, one per quadrant), issued at
    different PE row-strips so they overlap in the array.
  - Products: DVE tensor_mul reading the rotated tile straight from PSUM for
    4 tiles; tile 0 goes via an ACT copy so GpSimd multiplies it, keeping
    DVE below the PE window period.
  - mm2 = 5 K=128 bf16 chunks with host-side permuted+zero-padded w2
    (GpSimd-produced chunk accumulated last); fused BN2+ReLU on ACT.
  - 2-deep software pipeline: PE stream per window is
    mm1(w) | mm2_m0(w-2) | sel(w) | mm2_m1(w-2), so BN1/products hide
    behind matmuls and the product queue has a full window of slack;
    DMA-free warmup matmuls (memset tile) open the HAM clock-gate during
    the preamble, and the first x quarters load via the ACT hardware DGE
    in parallel with SP's const DMAs.
All weights are preprocessed host-side; pair order is folded into w2.
"""

import numpy as np

import concourse.bass as bass
import concourse.mybir as mybir
from concourse import tile
from concourse.bass_utils import run_bass_kernel_spmd

F32 = mybir.dt.float32
F32R = mybir.dt.float32r
BF16 = mybir.dt.bfloat16
AF = mybir.ActivationFunctionType

N_CORES = 8
B, CIN, H, W = 16, 256, 64, 64
NPIX = H * W                     # 4096
IMG_PER_CORE = B // N_CORES      # 2
CMID = 32
COUT = 256
FB = 512                         # pixel window (psum-bank sized)
NWIN = NPIX // FB                # 8 windows per image
EPS = 1e-5

# rotation sets per product tile (quadrant q of tile j uses ROTS[j][q])
ROTS = [[0, 1, 2, 3], [4, 5, 6, 7], [8, 9, 10, 11], [12, 13, 14, 15], [16, 0, 0, 0]]

_ctr = [0]


def _split_multi_waits(nc):
    """This container's walrus supports one sync-wait per instruction; split
    extras onto NOP carriers on the same engine."""
    for f in nc.m.functions:
        for blk in f.blocks:
            insts = blk.instructions
            if not any(
                i.sync_info is not None and len(i.sync_info.on_wait) > 1
                for i in insts
            ):
                continue
            new = []
            for inst in insts:
                si = inst.sync_info
                if si is not None and len(si.on_wait) > 1:
                    waits = list(si.on_wait)
                    for wcond in waits[:-1]:
                        _ctr[0] += 1
                        nop = mybir.InstNoOp(name=f"waitnop-{_ctr[0]}", ins=[], outs=[])
                        nop.engine = inst.engine
                        nop.sync_info = mybir.SyncInfo(on_wait=[wcond], on_update=[])
                        new.append(nop)
                    inst.sync_info = mybir.SyncInfo(
                        on_wait=[waits[-1]], on_update=list(si.on_update)
                    )
                new.append(inst)
            blk.instructions = new


def _host_weights(w1, b1, g1, be1, m1, v1, w2, b2, g2, be2, m2, v2):
    """Precompute device weight layouts on the host."""
    # mm1 lhsT, M-replicated: w1t[k, 32q+c] = w1[c, k]
    w1t = np.zeros((CIN, 128), np.float32)
    for q in range(4):
        w1t[:, 32 * q : 32 * q + 32] = w1.T
    inv1 = g1 / np.sqrt(v1 + EPS)
    bn1s = np.tile(inv1, 4).reshape(128, 1).astype(np.float32)
    bn1b = np.tile(b1 * inv1 + be1 - m1 * inv1, 4).reshape(128, 1).astype(np.float32)

    # permutation lhsT for the 5 rotation tiles, replicated across 4 strips:
    # perm[32i + k, 128j + 32q + c] = 1 iff k == (c + ROTS[j][q]) % 32
    perm = np.zeros((128, 5 * 128), np.float32)
    for j in range(5):
        for q in range(4):
            r = ROTS[j][q]
            for c in range(32):
                k = (c + r) % 32
                for i in range(4):
                    perm[32 * i + k, 128 * j + 32 * q + c] = 1.0

    # w2 permuted into the 5x128 product-row order; duplicate slots zeroed.
    off = np.zeros(33, np.int64)
    for d in range(32):
        off[d + 1] = off[d] + (32 - d)
    assert off[32] == 528
    w2p = np.zeros((5 * 128, COUT), np.float32)
    used = np.zeros(528, bool)
    for j in range(5):
        for q in range(4):
            r = ROTS[j][q]
            if j == 4 and q > 0:
                continue  # spare quadrants: weights stay zero
            for c in range(32):
                if r == 16 and c >= 16:
                    continue  # duplicate half of rotation 16
                if c + r < 32:
                    d, b_lo = r, c
                else:
                    d, b_lo = 32 - r, c + r - 32
                t = off[d] + b_lo
                assert not used[t]
                used[t] = True
                w2p[128 * j + 32 * q + c, :] = w2[:, t]
    assert used.all()

    inv2 = g2 / np.sqrt(v2 + EPS)
    bn2s = inv2.reshape(2, 128).T.astype(np.float32).copy()   # [128, 2] col m
    bn2b = (b2 * inv2 + be2 - m2 * inv2).reshape(2, 128).T.astype(np.float32).copy()
    return w1t, bn1s, bn1b, perm, w2p, bn2s, bn2b


def _build_nc():
    nc = bass.Bass()
    x_d = nc.declare_dram_parameter("x", [IMG_PER_CORE, CIN, NPIX], BF16, isOutput=False)
    w1t_d = nc.declare_dram_parameter("w1t", [CIN, 128], BF16, isOutput=False)
    bn1s_d = nc.declare_dram_parameter("bn1s", [128, 1], F32, isOutput=False)
    bn1b_d = nc.declare_dram_parameter("bn1b", [128, 1], F32, isOutput=False)
    perm_d = nc.declare_dram_parameter("perm", [128, 5 * 128], BF16, isOutput=False)
    w2p_d = nc.declare_dram_parameter("w2p", [5 * 128, COUT], BF16, isOutput=False)
    bn2s_d = nc.declare_dram_parameter("bn2s", [128, 2], F32, isOutput=False)
    bn2b_d = nc.declare_dram_parameter("bn2b", [128, 2], F32, isOutput=False)
    out_d = nc.declare_dram_parameter("out", [IMG_PER_CORE, COUT, NPIX], F32, isOutput=True)

    with tile.TileContext(nc) as tc:
        with (
            tc.tile_pool(name="consts", bufs=1) as cpool,
            tc.tile_pool(name="xp", bufs=1) as xpool,
            tc.tile_pool(name="yp", bufs=3) as ypool,
            tc.tile_pool(name="y4p", bufs=4) as y4pool,
            tc.tile_pool(name="pp", bufs=16) as ppool,
            tc.tile_pool(name="zp", bufs=4) as zpool,
            tc.tile_pool(name="psy", bufs=1, space="PSUM") as psum_y,
            tc.tile_pool(name="pss", bufs=4, space="PSUM") as psum_sel,
            tc.tile_pool(name="psz", bufs=3, space="PSUM") as psum_z,
        ):
            w1a = cpool.tile([128, 128], BF16, tag="w1a")
            w1b = cpool.tile([128, 128], BF16, tag="w1b")
            nc.sync.dma_start(w1a[:], w1t_d[0:128, :])
            nc.sync.dma_start(w1b[:], w1t_d[128:256, :])
            bn1s = cpool.tile([128, 1], F32, tag="bn1s")
            bn1b = cpool.tile([128, 1], F32, tag="bn1b")
            nc.sync.dma_start(bn1s[:], bn1s_d[:])
            nc.sync.dma_start(bn1b[:], bn1b_d[:])

            QRT = NPIX // 4
            xtiles = {}

            def load_x(img, h, split=1, eng=None):
                eng = eng or nc.sync
                xa = xpool.tile([128, QRT], BF16, tag=f"xa{img}{h}")
                xb = xpool.tile([128, QRT], BF16, tag=f"xb{img}{h}")
                step = QRT // split
                for p in range(split):
                    sl = slice(h * QRT + p * step, h * QRT + (p + 1) * step)
                    sd = slice(p * step, (p + 1) * step)
                    eng.dma_start(xa[:, sd], x_d[img, 0:128, sl])
                    eng.dma_start(xb[:, sd], x_d[img, 128:256, sl])
                xtiles[(img, h)] = (xa, xb)

            # first two quarters ride the ACT hardware DGE: issued during the
            # ACT preamble idle, in parallel with SP's const DMAs; the rest
            # stay on SP so they don't queue behind ACT compute.
            # first quarter as two single-window tile pairs: window 0's mm1
            # unblocks after only 256 KB of DMA
            w0tiles = []
            for wsub in range(2):
                xa = xpool.tile([128, FB], BF16, tag=f"xw{wsub}a")
                xb = xpool.tile([128, FB], BF16, tag=f"xw{wsub}b")
                sl = slice(wsub * FB, (wsub + 1) * FB)
                nc.scalar.dma_start(xa[:], x_d[0, 0:128, sl])
                nc.scalar.dma_start(xb[:], x_d[0, 128:256, sl])
                w0tiles.append((xa, xb))
            load_x(0, 1, eng=nc.scalar)

            # Warm the PE clock gate (HAM) while the first x tiles stream in.
            # Memset scratch instead of a DMA-loaded tile: the warmup matmuls
            # then have no DMA dependency and start right after the preamble.
            wz = cpool.tile([128, 128], BF16, tag="warmz")
            nc.vector.memset(wz[:], 0.0)
            ps_warm = psum_y.tile([128, FB], F32, tag="psy")
            for _ in range(44):
                nc.tensor.matmul(
                    ps_warm[:, 0:128], wz[:], wz[:], start=True, stop=True
                )

            perm_sb = cpool.tile([128, 5 * 128], BF16, tag="perm")
            nc.sync.dma_start(perm_sb[:], perm_d[:])
            bn2s = cpool.tile([128, 2], F32, tag="bn2s")
            bn2b = cpool.tile([128, 2], F32, tag="bn2b")
            nc.sync.dma_start(bn2s[:], bn2s_d[:])
            nc.sync.dma_start(bn2b[:], bn2b_d[:])
            w2p_sb = cpool.tile([128, 5 * COUT], BF16, tag="w2p")
            for j in range(5):
                nc.sync.dma_start(
                    w2p_sb[:, j * COUT : (j + 1) * COUT],
                    w2p_d[j * 128 : (j + 1) * 128, :],
                )
            load_x(0, 0)   # windows 2-3 of img0 read the tail of quarter 0
            load_x(0, 2)
            load_x(0, 3)
            load_x(1, 0)
            load_x(1, 1)
            load_x(1, 2)
            load_x(1, 3)

            def stage_a1(img, win):
                """mm1 + BN1 for one window."""
                if img == 0 and win < 2:
                    xa, xb = w0tiles[win]
                    s_loc = slice(0, FB)
                else:
                    h, wl = divmod(win, NWIN // 4)
                    s_loc = slice(wl * FB, (wl + 1) * FB)
                    xa, xb = xtiles[(img, h)]
                ps_y = psum_y.tile([128, FB], F32, tag="psy")
                nc.tensor.matmul(ps_y[:], w1a[:], xa[:, s_loc], start=True, stop=False)
                nc.tensor.matmul(ps_y[:], w1b[:], xb[:, s_loc], start=False, stop=True)
                yrep = ypool.tile([128, FB], BF16, tag="yrep")
                nc.scalar.activation(
                    yrep[:], ps_y[:], AF.Relu, bias=bn1b[:, 0:1], scale=bn1s[:, 0:1]
                )
                return yrep

            def stage_a2(yrep):
                """rotations + products for one window."""
                prods = {}
                for j in (0, 1, 2, 3, 4):
                    i = j % 4
                    ps_sel = psum_sel.tile([128, FB], F32, tag="pssel")
                    nc.tensor.matmul(
                        ps_sel[:],
                        perm_sb[32 * i : 32 * i + 32, 128 * j : 128 * (j + 1)],
                        yrep[32 * i : 32 * i + 32, :],
                        start=True,
                        stop=True,
                        tile_position=(32 * i, 0),
                    )
                    pj = ppool.tile([128, FB], BF16, tag="pj")
                    if j == 0:
                        # one product via ACT copy + GpSimd: keeps DVE below
                        # the PE window period so it never paces the pipeline
                        y4 = y4pool.tile([128, FB], BF16, tag="y4")
                        nc.scalar.activation(y4[:], ps_sel[:], AF.Copy)
                        nc.gpsimd.tensor_mul(pj[:], yrep[:], y4[:])
                    else:
                        # DVE reads the rotated tile straight from PSUM
                        nc.vector.tensor_mul(pj[:], yrep[:], ps_sel[:])
                    prods[j] = pj
                return [prods[j] for j in range(5)]

            def stage_b_m(img, win, prods, m):
                """mm2 m-chunk + BN2 + store for one window."""
                s = slice(win * FB, (win + 1) * FB)
                J_ORDER = (1, 2, 3, 4, 0)  # GpSimd-produced chunk last
                ps_z = psum_z.tile([128, FB], F32, tag="psz")
                for idx, j in enumerate(J_ORDER):
                    nc.tensor.matmul(
                        ps_z[:],
                        w2p_sb[:, j * COUT + 128 * m : j * COUT + 128 * m + 128],
                        prods[j][:],
                        start=(idx == 0),
                        stop=(idx == 4),
                    )
                zt = zpool.tile([128, FB], F32, tag="zt")
                nc.scalar.activation(
                    zt[:], ps_z[:], AF.Relu,
                    bias=bn2b[:, m : m + 1], scale=bn2s[:, m : m + 1],
                )
                nc.sync.dma_start(out_d[img, 128 * m : 128 * m + 128, s], zt[:])

            # software pipeline: PE stream per window is
            #   mm1(w) | mm2(w-1) | sel(w)
            # so BN1(w) (ACT) and products(w-1) (DVE/GP) hide behind mm2/sel
            # and the PE matmuls stay back-to-back.
            # 2-deep pipeline: mm2 consumes products from two windows back,
            # so the DVE/GpSimd product queue always has a full window of slack.
            pipe = []
            for img in range(IMG_PER_CORE):
                for win in range(NWIN):
                    yrep = stage_a1(img, win)
                    if len(pipe) == 2:
                        stage_b_m(*pipe[0], 0)
                    prods = stage_a2(yrep)
                    if len(pipe) == 2:
                        stage_b_m(*pipe.pop(0), 1)
                    pipe.append((img, win, prods))
            for ent in pipe:
                stage_b_m(*ent, 0)
                stage_b_m(*ent, 1)

    _split_multi_waits(nc)
    return nc


_cached = {}


def kernel(**inputs):
    x = np.ascontiguousarray(np.asarray(inputs["x"], np.float32))
    args = [
        np.asarray(inputs[k], np.float32)
        for k in ("w1", "b1", "g1", "be1", "m1", "v1", "w2", "b2", "g2", "be2", "m2", "v2")
    ]
    w1t, bn1s, bn1b, perm, w2p, bn2s, bn2b = _host_weights(*args)

    import ml_dtypes
    w2p = w2p.astype(ml_dtypes.bfloat16)
    w1t = w1t.astype(ml_dtypes.bfloat16)
    perm = perm.astype(ml_dtypes.bfloat16)
    if "nc" not in _cached:
        _cached["nc"] = _build_nc()
    nc = _cached["nc"]

    import ml_dtypes as _mld
    xr = x.reshape(B, CIN, NPIX).astype(_mld.bfloat16)
    shared = {
        "w1t": w1t, "bn1s": bn1s, "bn1b": bn1b, "perm": perm,
        "w2p": w2p, "bn2s": bn2s, "bn2b": bn2b,
    }
    in_maps = [
        {"x": np.ascontiguousarray(xr[c * IMG_PER_CORE : (c + 1) * IMG_PER_CORE]), **shared}
        for c in range(N_CORES)
    ]
    res = run_bass_kernel_spmd(nc, in_maps, core_ids=list(range(N_CORES)))
    kernel.last_results = res
    out = np.concatenate([res.results[c]["out"] for c in range(N_CORES)], axis=0)
    return out.reshape(B, COUT, H, W)


# revision 25
# speedup vs baseline: 1.1943x; 1.1943x over previous
"""Trainium2 Bass kernel for nn_BilinearModule (16,256,64,64 bilinear pooling).

Math (per image):
  y   = relu(bn1(w1 @ x + b1))                       # (32, 4096)
  packed[t] = y[r_t] * y[c_t]  for 528 lower-tri pairs
  out = relu(bn2(w2 @ packed + b2))                  # (256, 4096)

Strategy (pure data parallel over batch, 2 images per core, 8 cores):
  - all matmul operands bf16 (x cast host-side, halves the input DMA);
    fp32 PSUM accumulation and fp32 BN math keep the error ~5e-3.
  - mm1 with M-replicated weights -> psum; fused BN1+ReLU on ACT -> yrep bf16
    (4 identical copies of the 32 channels across 128 partitions).
  - The 528 pair-products are covered by 17 channel *rotations* r=0..16:
    rotation r yields pairs {c, (c+r)%32} = diag r plus diag 32-r, all
    distinct (r=16 half-duplicated). Rotated tiles are produced by 5 K=32
    permutation matmuls (4 rotations per tile, one per quadrant), issued at
    different PE row-strips so they overlap in the array.
  - Products: DVE tensor_mul reading the rotated tile straight from PSUM for
    4 tiles; tile 0 goes via an ACT copy so GpSimd multiplies it, keeping
    DVE below the PE window period.
  - mm2 = 5 K=128 bf16 chunks with host-side permuted+zero-padded w2
    (GpSimd-produced chunk accumulated last); fused BN2+ReLU on ACT.
  - 2-deep software pipeline: PE stream per window is
    mm1(w) | mm2_m0(w-2) | sel(w) | mm2_m1(w-2), so BN1/products hide
    behind matmuls and the product queue has a full window of slack;
    DMA-free warmup matmuls (memset tile) open the HAM clock-gate during
    the preamble, and the first x quarters load via the ACT hardware DGE
    in parallel with SP's const DMAs.
All weights are preprocessed host-side; pair order is folded into w2.
"""

import numpy as np

import concourse.bass as bass
import concourse.mybir as mybir
from concourse import tile
from concourse.bass_utils import run_bass_kernel_spmd

F32 = mybir.dt.float32
F32R = mybir.dt.float32r
BF16 = mybir.dt.bfloat16
AF = mybir.ActivationFunctionType

N_CORES = 8
B, CIN, H, W = 16, 256, 64, 64
NPIX = H * W                     # 4096
IMG_PER_CORE = B // N_CORES      # 2
CMID = 32
COUT = 256
FB = 512                         # pixel window (psum-bank sized)
NWIN = NPIX // FB                # 8 windows per image
EPS = 1e-5

# rotation sets per product tile (quadrant q of tile j uses ROTS[j][q])
ROTS = [[0, 1, 2, 3], [4, 5, 6, 7], [8, 9, 10, 11], [12, 13, 14, 15], [16, 0, 0, 0]]

_ctr = [0]


def _split_multi_waits(nc):
    """This container's walrus supports one sync-wait per instruction; split
    extras onto NOP carriers on the same engine."""
    for f in nc.m.functions:
        for blk in f.blocks:
            insts = blk.instructions
            if not any(
                i.sync_info is not None and len(i.sync_info.on_wait) > 1
                for i in insts
            ):
                continue
            new = []
            for inst in insts:
                si = inst.sync_info
                if si is not None and len(si.on_wait) > 1:
                    waits = list(si.on_wait)
                    for wcond in waits[:-1]:
                        _ctr[0] += 1
                        nop = mybir.InstNoOp(name=f"waitnop-{_ctr[0]}", ins=[], outs=[])
                        nop.engine = inst.engine
                        nop.sync_info = mybir.SyncInfo(on_wait=[wcond], on_update=[])
                        new.append(nop)
                    inst.sync_info = mybir.SyncInfo(
                        on_wait=[waits[-1]], on_update=list(si.on_update)
                    )
                new.append(inst)
            blk.instructions = new


def _host_weights(w1, b1, g1, be1, m1, v1, w2, b2, g2, be2, m2, v2):
    """Precompute device weight layouts on the host."""
    # mm1 lhsT, M-replicated: w1t[k, 32q+c] = w1[c, k]
    w1t = np.zeros((CIN, 128), np.float32)
    for q in range(4):
        w1t[:, 32 * q : 32 * q + 32] = w1.T
    inv1 = g1 / np.sqrt(v1 + EPS)
    bn1s = np.tile(inv1, 4).reshape(128, 1).astype(np.float32)
    bn1b = np.tile(b1 * inv1 + be1 - m1 * inv1, 4).reshape(128, 1).astype(np.float32)

    # permutation lhsT for the 5 rotation tiles, replicated across 4 strips:
    # perm[32i + k, 128j + 32q + c] = 1 iff k == (c + ROTS[j][q]) % 32
    perm = np.zeros((128, 5 * 128), np.float32)
    for j in range(5):
        for q in range(4):
            r = ROTS[j][q]
            for c in range(32):
                k = (c + r) % 32
                for i in range(4):
                    perm[32 * i + k, 128 * j + 32 * q + c] = 1.0

    # w2 permuted into the 5x128 product-row order; duplicate slots zeroed.
    off = np.zeros(33, np.int64)
    for d in range(32):
        off[d + 1] = off[d] + (32 - d)
    assert off[32] == 528
    w2p = np.zeros((5 * 128, COUT), np.float32)
    used = np.zeros(528, bool)
    for j in range(5):
        for q in range(4):
            r = ROTS[j][q]
            if j == 4 and q > 0:
                continue  # spare quadrants: weights stay zero
            for c in range(32):
                if r == 16 and c >= 16:
                    continue  # duplicate half of rotation 16
                if c + r < 32:
                    d, b_lo = r, c
                else:
                    d, b_lo = 32 - r, c + r - 32
                t = off[d] + b_lo
                assert not used[t]
                used[t] = True
                w2p[128 * j + 32 * q + c, :] = w2[:, t]
    assert used.all()

    inv2 = g2 / np.sqrt(v2 + EPS)
    bn2s = inv2.reshape(2, 128).T.astype(np.float32).copy()   # [128, 2] col m
    bn2b = (b2 * inv2 + be2 - m2 * inv2).reshape(2, 128).T.astype(np.float32).copy()
    return w1t, bn1s, bn1b, perm, w2p, bn2s, bn2b


def _build_nc():
    nc = bass.Bass()
    x_d = nc.declare_dram_parameter("x", [IMG_PER_CORE, CIN, NPIX], BF16, isOutput=False)
    w1t_d = nc.declare_dram_parameter("w1t", [CIN, 128], BF16, isOutput=False)
    bn1s_d = nc.declare_dram_parameter("bn1s", [128, 1], F32, isOutput=False)
    bn1b_d = nc.declare_dram_parameter("bn1b", [128, 1], F32, isOutput=False)
    perm_d = nc.declare_dram_parameter("perm", [128, 5 * 128], BF16, isOutput=False)
    w2p_d = nc.declare_dram_parameter("w2p", [5 * 128, COUT], BF16, isOutput=False)
    bn2s_d = nc.declare_dram_parameter("bn2s", [128, 2], F32, isOutput=False)
    bn2b_d = nc.declare_dram_parameter("bn2b", [128, 2], F32, isOutput=False)
    out_d = nc.declare_dram_parameter("out", [IMG_PER_CORE, COUT, NPIX], F32, isOutput=True)

    with tile.TileContext(nc) as tc:
        with (
            tc.tile_pool(name="consts", bufs=1) as cpool,
            tc.tile_pool(name="xp", bufs=1) as xpool,
            tc.tile_pool(name="yp", bufs=3) as ypool,
            tc.tile_pool(name="y4p", bufs=4) as y4pool,
            tc.tile_pool(name="pp", bufs=16) as ppool,
            tc.tile_pool(name="zp", bufs=4) as zpool,
            tc.tile_pool(name="psy", bufs=1, space="PSUM") as psum_y,
            tc.tile_pool(name="pss", bufs=4, space="PSUM") as psum_sel,
            tc.tile_pool(name="psz", bufs=3, space="PSUM") as psum_z,
        ):
            w1a = cpool.tile([128, 128], BF16, tag="w1a")
            w1b = cpool.tile([128, 128], BF16, tag="w1b")
            nc.sync.dma_start(w1a[:], w1t_d[0:128, :])
            nc.sync.dma_start(w1b[:], w1t_d[128:256, :])
            bn1s = cpool.tile([128, 1], F32, tag="bn1s")
            bn1b = cpool.tile([128, 1], F32, tag="bn1b")
            nc.sync.dma_start(bn1s[:], bn1s_d[:])
            nc.sync.dma_start(bn1b[:], bn1b_d[:])

            QRT = NPIX // 4
            xtiles = {}

            def load_x(img, h, split=1, eng=None):
                eng = eng or nc.sync
                xa = xpool.tile([128, QRT], BF16, tag=f"xa{img}{h}")
                xb = xpool.tile([128, QRT], BF16, tag=f"xb{img}{h}")
                step = QRT // split
                for p in range(split):
                    sl = slice(h * QRT + p * step, h * QRT + (p + 1) * step)
                    sd = slice(p * step, (p + 1) * step)
                    eng.dma_start(xa[:, sd], x_d[img, 0:128, sl])
                    eng.dma_start(xb[:, sd], x_d[img, 128:256, sl])
                xtiles[(img, h)] = (xa, xb)

            # first two quarters ride the ACT hardware DGE: issued during the
            # ACT preamble idle, in parallel with SP's const DMAs; the rest
            # stay on SP so they don't queue behind ACT compute.
            load_x(0, 0, split=2, eng=nc.scalar)
            load_x(0, 1, eng=nc.scalar)

            # Warm the PE clock gate (HAM) while the first x tiles stream in.
            # Memset scratch instead of a DMA-loaded tile: the warmup matmuls
            # then have no DMA dependency and start right after the preamble.
            wz = cpool.tile([128, 128], BF16, tag="warmz")
            nc.vector.memset(wz[:], 0.0)
            ps_warm = psum_y.tile([128, FB], F32, tag="psy")
            for _ in range(28):
                nc.tensor.matmul(
                    ps_warm[:, 0:128], wz[:], wz[:], start=True, stop=True
                )

            perm_sb = cpool.tile([128, 5 * 128], BF16, tag="perm")
            nc.sync.dma_start(perm_sb[:], perm_d[:])
            bn2s = cpool.tile([128, 2], F32, tag="bn2s")
            bn2b = cpool.tile([128, 2], F32, tag="bn2b")
            nc.sync.dma_start(bn2s[:], bn2s_d[:])
            nc.sync.dma_start(bn2b[:], bn2b_d[:])
            w2p_sb = cpool.tile([128, 5 * COUT], BF16, tag="w2p")
            for j in range(5):
                nc.sync.dma_start(
                    w2p_sb[:, j * COUT : (j + 1) * COUT],
                    w2p_d[j * 128 : (j + 1) * 128, :],
                )
            load_x(0, 2)
            load_x(0, 3)
            load_x(1, 0)
            load_x(1, 1)
            load_x(1, 2)
            load_x(1, 3)

            def stage_a1(img, win):
                """mm1 + BN1 for one window."""
                h, wl = divmod(win, NWIN // 4)
                s_loc = slice(wl * FB, (wl + 1) * FB)
                xa, xb = xtiles[(img, h)]
                ps_y = psum_y.tile([128, FB], F32, tag="psy")
                nc.tensor.matmul(ps_y[:], w1a[:], xa[:, s_loc], start=True, stop=False)
                nc.tensor.matmul(ps_y[:], w1b[:], xb[:, s_loc], start=False, stop=True)
                yrep = ypool.tile([128, FB], BF16, tag="yrep")
                nc.scalar.activation(
                    yrep[:], ps_y[:], AF.Relu, bias=bn1b[:, 0:1], scale=bn1s[:, 0:1]
                )
                return yrep

            def stage_a2(yrep):
                """rotations + products for one window."""
                prods = {}
                for j in (0, 1, 2, 3, 4):
                    i = j % 4
                    ps_sel = psum_sel.tile([128, FB], F32, tag="pssel")
                    nc.tensor.matmul(
                        ps_sel[:],
                        perm_sb[32 * i : 32 * i + 32, 128 * j : 128 * (j + 1)],
                        yrep[32 * i : 32 * i + 32, :],
                        start=True,
                        stop=True,
                        tile_position=(32 * i, 0),
                    )
                    pj = ppool.tile([128, FB], BF16, tag="pj")
                    if j == 0:
                        # one product via ACT copy + GpSimd: keeps DVE below
                        # the PE window period so it never paces the pipeline
                        y4 = y4pool.tile([128, FB], BF16, tag="y4")
                        nc.scalar.activation(y4[:], ps_sel[:], AF.Copy)
                        nc.gpsimd.tensor_mul(pj[:], yrep[:], y4[:])
                    else:
                        # DVE reads the rotated tile straight from PSUM
                        nc.vector.tensor_mul(pj[:], yrep[:], ps_sel[:])
                    prods[j] = pj
                return [prods[j] for j in range(5)]

            def stage_b_m(img, win, prods, m):
                """mm2 m-chunk + BN2 + store for one window."""
                s = slice(win * FB, (win + 1) * FB)
                J_ORDER = (1, 2, 3, 4, 0)  # GpSimd-produced chunk last
                ps_z = psum_z.tile([128, FB], F32, tag="psz")
                for idx, j in enumerate(J_ORDER):
                    nc.tensor.matmul(
                        ps_z[:],
                        w2p_sb[:, j * COUT + 128 * m : j * COUT + 128 * m + 128],
                        prods[j][:],
                        start=(idx == 0),
                        stop=(idx == 4),
                    )
                zt = zpool.tile([128, FB], F32, tag="zt")
                nc.scalar.activation(
                    zt[:], ps_z[:], AF.Relu,
                    bias=bn2b[:, m : m + 1], scale=bn2s[:, m : m + 1],
                )
                nc.sync.dma_start(out_d[img, 128 * m : 128 * m + 128, s], zt[:])

            # software pipeline: PE stream per window is
            #   mm1(w) | mm2(w-1) | sel(w)
            # so BN1(w) (ACT) and products(w-1) (DVE/GP) hide behind mm2/sel
            # and the PE matmuls stay back-to-back.
            # 2-deep pipeline: mm2 consumes products from two windows back,
            # so the DVE/GpSimd product queue always has a full window of slack.
            pipe = []
            for img in range(IMG_PER_CORE):
                for win in range(NWIN):
                    yrep = stage_a1(img, win)
                    if len(pipe) == 2:
                        stage_b_m(*pipe[0], 0)
                    prods = stage_a2(yrep)
                    if len(pipe) == 2:
                        stage_b_m(*pipe.pop(0), 1)
                    pipe.append((img, win, prods))
            for ent in pipe:
                stage_b_m(*ent, 0)
                stage_b_m(*ent, 1)

    _split_multi_waits(nc)
    return nc


_cached = {}


def kernel(**inputs):
    x = np.ascontiguousarray(np.asarray(inputs["x"], np.float32))
    args = [
        np.asarray(inputs[k], np.float32)
        for k in ("w1", "b1", "g1", "be1", "m1", "v1", "w2", "b2", "g2", "be2", "m2", "v2")
    ]
    w1t, bn1s, bn1b, perm, w2p, bn2s, bn2b = _host_weights(*args)

    import ml_dtypes
    w2p = w2p.astype(ml_dtypes.bfloat16)
    w1t = w1t.astype(ml_dtypes.bfloat16)
    perm = perm.astype(ml_dtypes.bfloat16)
    if "nc" not in _cached:
        _cached["nc"] = _build_nc()
    nc = _cached["nc"]

    import ml_dtypes as _mld
    xr = x.reshape(B, CIN, NPIX).astype(_mld.bfloat16)
    shared = {
        "w1t": w1t, "bn1s": bn1s, "bn1b": bn1b, "perm": perm,
        "w2p": w2p, "bn2s": bn2s, "bn2b": bn2b,
    }
    in_maps = [
        {"x": np.ascontiguousarray(xr[c * IMG_PER_CORE : (c + 1) * IMG_PER_CORE]), **shared}
        for c in range(N_CORES)
    ]
    res = run_bass_kernel_spmd(nc, in_maps, core_ids=list(range(N_CORES)))
    kernel.last_results = res
    out = np.concatenate([res.results[c]["out"] for c in range(N_CORES)], axis=0)
    return out.reshape(B, COUT, H, W)
